# revision 4
# baseline (speedup 1.0000x reference)
"""3x3 valid cross-correlation of a 4096x4096 fp32 image + scalar bias,
sharded row-wise across 8 TRN2 NeuronCores.

Strategy per core (512 output rows, 514 input rows incl. 2-row halo taken
host-side via overlapping slices -- no device collectives):
  - Row panels of 128 input rows -> 126 output rows (banded matmul):
    out[m, n] = sum_dc sum_dr w[dr, dc] * x[m+dr, n+dc]
    For each kernel column dc, a banded stationary matrix
    B_dc[k, m] = w[k-m, dc] (k-m in 0..2) gives
    (B_dc.T-free) matmul: psum[m, n] += sum_k B_dc[k, m] * x[k, n+dc].
    The 3 dc-matmuls accumulate into one PSUM bank; the column shift dc is
    folded into the moving-operand (rhs) free-dim offset.
  - Bias is fused into the PSUM->SBUF copy via ScalarE activation bias.
  - 4 full panels (126 rows) + 1 tail panel (10 input rows -> 8 output rows)
    cover 512 output rows. Last core overlaps core 6 by 2 rows so that all
    cores run an identical 514-row program (4094 = 8*512 - 2).
"""

import numpy as np

import concourse.bacc as bacc
import concourse.mybir as mybir
from concourse import tile
from concourse.bass_utils import run_bass_kernel_spmd

H, W = 4096, 4096
KH, KW = 3, 3
OH, OW = H - KH + 1, W - KW + 1  # 4094, 4094
NCORES = 8
ROWS_PER_CORE = 512              # output rows computed per core
IN_ROWS = ROWS_PER_CORE + KH - 1  # 514 input rows per core
PANEL_OUT = 126                  # output rows per full 128-input-row panel
N_FULL_PANELS = 4                # 4 * 126 = 504
TAIL_OUT = ROWS_PER_CORE - N_FULL_PANELS * PANEL_OUT  # 8
TAIL_IN = TAIL_OUT + KH - 1      # 10
COLS_PER_MM = 512                # fp32 moving-operand / PSUM-bank max

_F32 = mybir.dt.float32

_PROGRAM_CACHE = None
last_results = None  # BassKernelResults of the most recent kernel() call


def _build_program():
    nc = bacc.Bacc(
        "TRN2", target_bir_lowering=False, debug=False, num_devices=NCORES
    )
    x = nc.dram_tensor("x", [IN_ROWS, W], _F32, kind="ExternalInput")
    w = nc.dram_tensor("w", [128, KW * PANEL_OUT], _F32, kind="ExternalInput")
    b = nc.dram_tensor("b", [128, 1], _F32, kind="ExternalInput")
    y = nc.dram_tensor("y", [ROWS_PER_CORE, OW], _F32, kind="ExternalOutput")

    n_col_tiles = (OW + COLS_PER_MM - 1) // COLS_PER_MM  # 8

    with tile.TileContext(nc) as tc:
        with (
            tc.tile_pool(name="const", bufs=1) as cpool,
            tc.tile_pool(name="xp", bufs=3) as xpool,
            tc.tile_pool(name="op", bufs=2) as opool,
            tc.tile_pool(name="pp", bufs=4, space="PSUM") as ppool,
        ):
            wt = cpool.tile([128, KW * PANEL_OUT], _F32)
            nc.sync.dma_start(wt[:], w[:])
            bt = cpool.tile([128, 1], _F32)
            nc.sync.dma_start(bt[:], b[:])

            for panel in range(N_FULL_PANELS + 1):
                r0 = PANEL_OUT * panel
                K = 128 if panel < N_FULL_PANELS else TAIL_IN
                M = PANEL_OUT if panel < N_FULL_PANELS else TAIL_OUT

                xt = xpool.tile([128, W], _F32)
                nc.sync.dma_start(xt[:K, :], x[r0 : r0 + K, :])

                ot = opool.tile([128, OW], _F32)
                for j in range(n_col_tiles):
                    c0 = j * COLS_PER_MM
                    N = min(COLS_PER_MM, OW - c0)
                    ps = ppool.tile([128, COLS_PER_MM], _F32)
                    for dc in range(KW):
                        nc.tensor.matmul(
                            ps[:M, :N],
                            wt[:K, dc * PANEL_OUT : dc * PANEL_OUT + M],
                            xt[:K, c0 + dc : c0 + dc + N],
                            start=(dc == 0),
                            stop=(dc == KW - 1),
                        )
                    nc.scalar.activation(
                        ot[:M, c0 : c0 + N],
                        ps[:M, :N],
                        mybir.ActivationFunctionType.Identity,
                        bias=bt[:M, :],
                    )
                nc.sync.dma_start(y[r0 : r0 + M, :], ot[:M, :])
    nc.compile()
    return nc


def _banded_weights(weight: np.ndarray) -> np.ndarray:
    """lhsT for each kernel column dc, laid out as [128, KW*PANEL_OUT].

    wT[k, dc*PANEL_OUT + m] = weight[k - m, dc] for 0 <= k - m < KH.
    The tail panel's [TAIL_IN, TAIL_OUT] banded matrix is the top-left
    block of the same layout, so one tensor serves both panel shapes.
    """
    wT = np.zeros((128, KW * PANEL_OUT), np.float32)
    m = np.arange(PANEL_OUT)
    for dc in range(KW):
        for d in range(KH):
            wT[m + d, dc * PANEL_OUT + m] = weight[d, dc]
    return wT


def _install_ntff_hook():
    """Shim antenv.axon_hooks so run_bass_kernel_spmd(trace=True) can find
    the axon NTFF profiling hook (the image's antenv lacks axon_hooks)."""
    import sys
    import types

    try:
        from antenv.axon_hooks import get_axon_ntff_profile_hook  # noqa: F401

        return
    except ImportError:
        pass
    import antenv
    from trn_agent_boot.trn_boot import _ntff_profile_via_ctypes

    hook = _ntff_profile_via_ctypes("/opt/axon/libaxon_pjrt.so")
    mod = types.ModuleType("antenv.axon_hooks")
    mod._hook = hook
    mod.set_axon_ntff_profile_hook = lambda h: setattr(mod, "_hook", h)
    mod.get_axon_ntff_profile_hook = lambda: mod._hook
    sys.modules["antenv.axon_hooks"] = mod
    antenv.axon_hooks = mod


def kernel(x, weight, bias, _trace=False, _trace_cores=None):
    global _PROGRAM_CACHE, last_results
    if _trace:
        _install_ntff_hook()
    x = np.ascontiguousarray(np.asarray(x, dtype=np.float32))
    weight = np.asarray(weight, dtype=np.float32)
    bias = np.asarray(bias, dtype=np.float32)

    if _PROGRAM_CACHE is None:
        _PROGRAM_CACHE = _build_program()
    nc = _PROGRAM_CACHE

    wT = _banded_weights(weight)
    bb = np.full((128, 1), bias[0], np.float32)

    in_maps = []
    for i in range(NCORES):
        r0 = i * ROWS_PER_CORE if i < NCORES - 1 else H - IN_ROWS
        in_maps.append(
            {"x": np.ascontiguousarray(x[r0 : r0 + IN_ROWS]), "w": wT, "b": bb}
        )

    kwargs = {}
    if _trace:
        kwargs["trace"] = True
        kwargs["trace_cores"] = (
            list(range(NCORES)) if _trace_cores is None else _trace_cores
        )
    res = run_bass_kernel_spmd(nc, in_maps, core_ids=list(range(NCORES)), **kwargs)
    last_results = res

    out = np.empty((OH, OW), np.float32)
    for i in range(NCORES - 1):
        out[i * ROWS_PER_CORE : (i + 1) * ROWS_PER_CORE] = res.results[i]["y"]
    tail_rows = OH - (NCORES - 1) * ROWS_PER_CORE  # 510
    out[(NCORES - 1) * ROWS_PER_CORE :] = res.results[-1]["y"][
        ROWS_PER_CORE - tail_rows :
    ]
    return out


# revision 7
# speedup vs baseline: 1.9427x; 1.9427x over previous
"""3x3 valid cross-correlation of a 4096x4096 fp32 image + scalar bias,
sharded row-wise across 8 TRN2 NeuronCores.

Strategy per core (512 output rows, 514 input rows incl. 2-row halo taken
host-side via overlapping slices -- no device collectives):
  - Row panels of 128 input rows -> 126 output rows (banded matmul):
    out[m, n] = sum_dc sum_dr w[dr, dc] * x[m+dr, n+dc]
    For each kernel column dc, a banded stationary matrix
    B_dc[k, m] = w[k-m, dc] (k-m in 0..2) gives
    (B_dc.T-free) matmul: psum[m, n] += sum_k B_dc[k, m] * x[k, n+dc].
    The 3 dc-matmuls accumulate into one PSUM bank; the column shift dc is
    folded into the moving-operand (rhs) free-dim offset.
  - Bias is fused into the PSUM->SBUF copy via ScalarE activation bias.
  - 4 full panels (126 rows) + 1 tail panel (10 input rows -> 8 output rows)
    cover 512 output rows. Last core overlaps core 6 by 2 rows so that all
    cores run an identical 514-row program (4094 = 8*512 - 2).
"""

import numpy as np

import concourse.bacc as bacc
import concourse.mybir as mybir
from concourse import tile
from concourse.bass_utils import run_bass_kernel_spmd

H, W = 4096, 4096
KH, KW = 3, 3
OH, OW = H - KH + 1, W - KW + 1  # 4094, 4094
NCORES = 8
ROWS_PER_CORE = 512              # output rows computed per core
IN_ROWS = ROWS_PER_CORE + KH - 1  # 514 input rows per core
PANEL_OUT = 126                  # output rows per full 128-input-row panel
N_FULL_PANELS = 4                # 4 * 126 = 504
TAIL_OUT = ROWS_PER_CORE - N_FULL_PANELS * PANEL_OUT  # 8
TAIL_IN = TAIL_OUT + KH - 1      # 10
COLS_PER_MM = 512                # fp32 moving-operand / PSUM-bank max

_F32 = mybir.dt.float32
_F32R = mybir.dt.float32r

_PROGRAM_CACHE = None
last_results = None  # BassKernelResults of the most recent kernel() call


def _build_program():
    nc = bacc.Bacc(
        "TRN2", target_bir_lowering=False, debug=False, num_devices=NCORES
    )
    x = nc.dram_tensor("x", [IN_ROWS, W], _F32, kind="ExternalInput")
    w = nc.dram_tensor("w", [128, KW * PANEL_OUT], _F32, kind="ExternalInput")
    b = nc.dram_tensor("b", [128, 1], _F32, kind="ExternalInput")
    y = nc.dram_tensor("y", [ROWS_PER_CORE, OW], _F32, kind="ExternalOutput")

    n_col_tiles = (OW + COLS_PER_MM - 1) // COLS_PER_MM  # 8

    with tile.TileContext(nc) as tc:
        with (
            tc.tile_pool(name="const", bufs=1) as cpool,
            tc.tile_pool(name="xp", bufs=2) as xpool,
            tc.tile_pool(name="xr", bufs=2) as xrpool,
            tc.tile_pool(name="op", bufs=2) as opool,
            tc.tile_pool(name="pp", bufs=4, space="PSUM") as ppool,
        ):
            wt = cpool.tile([128, KW * PANEL_OUT], _F32)
            nc.sync.dma_start(wt[:], w[:])
            bt = cpool.tile([128, 1], _F32)
            nc.sync.dma_start(bt[:], b[:])
            # fp32r operands must come from an instruction that rounds to
            # fp32r precision (walrus checkMatmultFP32r), so bounce both
            # matmul operands through a converting copy.
            wtr = cpool.tile([128, KW * PANEL_OUT], _F32R)
            nc.vector.tensor_copy(wtr[:], wt[:])

            for panel in range(N_FULL_PANELS + 1):
                r0 = PANEL_OUT * panel
                K = 128 if panel < N_FULL_PANELS else TAIL_IN
                M = PANEL_OUT if panel < N_FULL_PANELS else TAIL_OUT

                xt = xpool.tile([128, W], _F32)
                nc.sync.dma_start(xt[:K, :], x[r0 : r0 + K, :])
                xtr = xrpool.tile([128, W], _F32R)
                nc.vector.tensor_copy(xtr[:K, :], xt[:K, :])

                ot = opool.tile([128, OW], _F32)
                for j in range(n_col_tiles):
                    c0 = j * COLS_PER_MM
                    N = min(COLS_PER_MM, OW - c0)
                    ps = ppool.tile([128, COLS_PER_MM], _F32)
                    for dc in range(KW):
                        # float32r: single-pass fp32 matmul (1 cycle/row at
                        # N>=256) vs float32's 2-pass LOW_HIGH at 4 cycles/row.
                        nc.tensor.matmul(
                            ps[:M, :N],
                            wtr[:K, dc * PANEL_OUT : dc * PANEL_OUT + M],
                            xtr[:K, c0 + dc : c0 + dc + N],
                            start=(dc == 0),
                            stop=(dc == KW - 1),
                        )
                    nc.scalar.activation(
                        ot[:M, c0 : c0 + N],
                        ps[:M, :N],
                        mybir.ActivationFunctionType.Identity,
                        bias=bt[:M, :],
                    )
                nc.sync.dma_start(y[r0 : r0 + M, :], ot[:M, :])
    nc.compile()
    return nc


def _banded_weights(weight: np.ndarray) -> np.ndarray:
    """lhsT for each kernel column dc, laid out as [128, KW*PANEL_OUT].

    wT[k, dc*PANEL_OUT + m] = weight[k - m, dc] for 0 <= k - m < KH.
    The tail panel's [TAIL_IN, TAIL_OUT] banded matrix is the top-left
    block of the same layout, so one tensor serves both panel shapes.
    """
    wT = np.zeros((128, KW * PANEL_OUT), np.float32)
    m = np.arange(PANEL_OUT)
    for dc in range(KW):
        for d in range(KH):
            wT[m + d, dc * PANEL_OUT + m] = weight[d, dc]
    return wT


def _install_ntff_hook():
    """Shim antenv.axon_hooks so run_bass_kernel_spmd(trace=True) can find
    the axon NTFF profiling hook (the image's antenv lacks axon_hooks)."""
    import sys
    import types

    try:
        from antenv.axon_hooks import get_axon_ntff_profile_hook  # noqa: F401

        return
    except ImportError:
        pass
    import antenv
    from trn_agent_boot.trn_boot import _ntff_profile_via_ctypes

    hook = _ntff_profile_via_ctypes("/opt/axon/libaxon_pjrt.so")
    mod = types.ModuleType("antenv.axon_hooks")
    mod._hook = hook
    mod.set_axon_ntff_profile_hook = lambda h: setattr(mod, "_hook", h)
    mod.get_axon_ntff_profile_hook = lambda: mod._hook
    sys.modules["antenv.axon_hooks"] = mod
    antenv.axon_hooks = mod


def kernel(x, weight, bias, _trace=False, _trace_cores=None):
    global _PROGRAM_CACHE, last_results
    if _trace:
        _install_ntff_hook()
    x = np.ascontiguousarray(np.asarray(x, dtype=np.float32))
    weight = np.asarray(weight, dtype=np.float32)
    bias = np.asarray(bias, dtype=np.float32)

    if _PROGRAM_CACHE is None:
        _PROGRAM_CACHE = _build_program()
    nc = _PROGRAM_CACHE

    wT = _banded_weights(weight)
    bb = np.full((128, 1), bias[0], np.float32)

    in_maps = []
    for i in range(NCORES):
        r0 = i * ROWS_PER_CORE if i < NCORES - 1 else H - IN_ROWS
        in_maps.append(
            {"x": np.ascontiguousarray(x[r0 : r0 + IN_ROWS]), "w": wT, "b": bb}
        )

    kwargs = {}
    if _trace:
        kwargs["trace"] = True
        kwargs["trace_cores"] = (
            list(range(NCORES)) if _trace_cores is None else _trace_cores
        )
    res = run_bass_kernel_spmd(nc, in_maps, core_ids=list(range(NCORES)), **kwargs)
    last_results = res

    out = np.empty((OH, OW), np.float32)
    for i in range(NCORES - 1):
        out[i * ROWS_PER_CORE : (i + 1) * ROWS_PER_CORE] = res.results[i]["y"]
    tail_rows = OH - (NCORES - 1) * ROWS_PER_CORE  # 510
    out[(NCORES - 1) * ROWS_PER_CORE :] = res.results[-1]["y"][
        ROWS_PER_CORE - tail_rows :
    ]
    return out
